# revision 40
# baseline (speedup 1.0000x reference)
"""Multi-head graph attention layer (GAT) for Trainium2, 8-core data-parallel.

Problem: B=8, N=1024, D_IN=256, D_OUT=64, H=8, LeakyReLU slope 0.2.
Sharding: one batch element per NeuronCore.

Algebra: with x = f1_i + f2_j and exp monotone, the unnormalized softmax
weight (after factoring out exp(0.2 f1_i), which cancels) is
  U[j,i] = adj[j,i] * max(d_i * E2_j, E2s_j)
with d = exp(0.8 f1), E2 = exp(f2), E2s = exp(0.2 f2).
out^T = [Wh|1]^T @ U gives numerators + the denominator row Z; the
finalize transposes via the DMA XBAR and normalizes.

Measured op costs (ns, on throttled shared hw): DVE TS [P,1024] ~540,
DVE mask TT ~550/unit (mega-quad [P,2,4,N]), ACT pass ~1100-1230, Pool
TT [P,2,N] ~4300 (useless for big tiles).  Hence: all masks on DVE
mega-quads; B lanes (2 ACT passes, jt4/5 + jt6k0) only where DVE
saturates; Pool does only the small normalize TTs.  Pair 0 is all-D so
ACT pre-computes pair 1's B tiles during it (pipeline fill).

Everything is bf16 (one h load; scores f32-accumulated on PE).  f2 is
computed directly as COLUMNS via tiny per-node-tile matmuls (no XBAR
transposes in the prologue); d rows bounce via DRAM and are partition-
broadcast in split halves across both HWDGE queues at high priority,
ahead of the bulk adj loads (the DMA fabric is the prologue
bottleneck).  Finalize (PSUM evac -> XBAR transpose -> reciprocal ->
normalize -> per-pair output DMA) is pipelined one pair behind and
runs at high priority so it interleaves with the loop instead of
serializing at the end.  Broadcast-view reads (mask TTs) do not
register DMA dependencies in the tile framework; tiny anchor copies on
the DVE queue order each adj tile before its first masked use.
"""

import numpy as np
import ml_dtypes

BF16 = ml_dtypes.bfloat16

B, N, D_IN, D_OUT, H = 8, 1024, 256, 64, 8
NEG_SLOPE = 0.2
P = 128
NJT = N // P                  # 8 j tiles
NIT = N // P                  # 8 i tiles
NKT = D_IN // P               # 2 contraction tiles
HF = H * D_OUT                # 512
AUG = D_OUT + 1               # 65
TRW = 80                      # transpose row count (65 padded to %16)
NPAIR = H // 2
W12C = 2 * H + 24             # [w2 | zero pad | w1], f1 rows at partition 32

# B-lane units per pair (pair 0 all-D): jt4,jt5 both k, jt6 k0
B_UNITS = {1: ((4, 0), (4, 1), (5, 0), (5, 1), (6, 0)),
           2: ((4, 0), (4, 1), (5, 0), (5, 1), (6, 0)),
           3: ((4, 0), (4, 1), (5, 0), (5, 1), (6, 0)),
           0: ()}


def _build_program():
    import concourse.bass as bass
    import concourse.bacc as bacc
    import concourse.tile as tile
    from concourse import mybir

    f32 = mybir.dt.float32
    bf16 = mybir.dt.bfloat16
    AF = mybir.ActivationFunctionType
    OP = mybir.AluOpType

    nc = bacc.Bacc("TRN2", target_bir_lowering=False, debug=False,
                   enable_asserts=False, num_devices=8)

    hTb = nc.dram_tensor("hTb", [D_IN, N], bf16, kind="ExternalInput").ap()
    adjT = nc.dram_tensor("adjT", [N, N], bf16, kind="ExternalInput").ap()
    wrsb = nc.dram_tensor("wrsb", [D_IN, HF], bf16,
                          kind="ExternalInput").ap()
    w12 = nc.dram_tensor("w12", [D_IN, W12C], bf16,
                         kind="ExternalInput").ap()
    out = nc.dram_tensor("out", [N, HF], bf16, kind="ExternalOutput").ap()

    with tile.TileContext(nc) as tc:
        with (
            tc.tile_pool(name="const", bufs=1) as const,
            tc.tile_pool(name="inputs", bufs=1) as inputs,
            tc.tile_pool(name="whp", bufs=1) as whp,
            tc.tile_pool(name="ecol", bufs=1) as ecolp,
            tc.tile_pool(name="ps_f", bufs=1, space="PSUM") as ps_f,
            tc.tile_pool(name="ps_misc", bufs=1, space="PSUM") as ps_misc,
            tc.tile_pool(name="ps_ot", bufs=2, space="PSUM") as ps_ot,
            tc.tile_pool(name="work", bufs=1) as work,
            tc.tile_pool(name="ump", bufs=2) as ump,
            tc.tile_pool(name="fin", bufs=2) as fin,
            tc.tile_pool(name="dram", bufs=1, space="DRAM") as dramp,
        ):
            # ---- Phase 0: DMA issue ----------------------------------------
            # SP: score/Wh inputs first, then adj3/4 while dTt pends
            htb_sb = inputs.tile([P, NKT, N], bf16)
            w12_sb = inputs.tile([P, NKT, W12C], bf16)
            wrs_sb = inputs.tile([P, NKT, HF], bf16)
            adj_all = inputs.tile([P, NJT, N], bf16)
            for kt in range(NKT):
                nc.sync.dma_start(out=htb_sb[:, kt, :],
                                  in_=hTb[kt * P:(kt + 1) * P, :])
            for kt in range(NKT):
                nc.scalar.dma_start(out=w12_sb[:, kt, :],
                                    in_=w12[kt * P:(kt + 1) * P, :])

            # Pool: warmup operand + whaug ones + persistent evac targets
            z512 = const.tile([P, 512], bf16)
            nc.gpsimd.memset(z512, 0.0)
            whaug = []
            for it in range(NIT):
                wa = whp.tile([P, H, AUG], bf16, tag=f"whaug{it}",
                              name=f"whaug{it}")
                nc.gpsimd.memset(wa[:, :, D_OUT], 1.0)
                whaug.append(wa)
            ots_tiles = []
            for i in range(4):
                t = whp.tile([TRW, N], bf16, tag=f"ots{i}", name=f"ots{i}")
                nc.gpsimd.memset(t[D_OUT:TRW, :], 0.0)
                ots_tiles.append(t)

            # ---- PE warmup chain (p-state ramp), reusing the score PSUM ----
            fps = ps_f.tile([H, N], f32)
            NWARM = 7
            for i in range(NWARM):
                nc.tensor.matmul(fps[:, 0:512], z512[:, 0:H], z512,
                                 start=(i == 0), stop=(i == NWARM - 1))

            # ---- Phase 1: f1 score rows (w1 block of w12) ------------------
            for half in range(2):
                sl = slice(half * 512, (half + 1) * 512)
                for kt in range(NKT):
                    nc.tensor.matmul(fps[:, sl], w12_sb[:, kt, 32:32 + H],
                                     htb_sb[:, kt, sl],
                                     start=(kt == 0), stop=(kt == NKT - 1))

            # ---- f2 as columns: tiny matmuls, node tiles stationary --------
            fcols = ps_f.tile([P, NIT, H], f32, tag="fc")
            for it in range(NIT):
                for kt in range(NKT):
                    nc.tensor.matmul(fcols[:, it, :],
                                     htb_sb[:, kt, it * P:(it + 1) * P],
                                     w12_sb[:, kt, 0:H],
                                     start=(kt == 0), stop=(kt == NKT - 1))

            # d = exp(0.8 f1) row form -> DRAM bounce -> partition
            # broadcast; E2/E2s straight from the f2 COLUMNS (no XBAR
            # transposes).  This chain gates the first attention op: pin
            # it at high priority.
            dTt = ecolp.tile([H, N], bf16)
            dT_dram = dramp.tile([H, N], bf16)
            dbc_all = ecolp.tile([P, H, N], bf16)
            ec_all = ecolp.tile([P, NIT, 3 * H], f32)

            def bcast(h, eng=None):
                src_ap = dT_dram[h:h + 1, :]
                nc.sync.dma_start(
                    out=dbc_all[:, h, 0:512],
                    in_=src_ap[:, 0:512].partition_broadcast(P))
                nc.scalar.dma_start(
                    out=dbc_all[:, h, 512:1024],
                    in_=src_ap[:, 512:1024].partition_broadcast(P))

            with tc.high_priority():
                nc.scalar.activation(dTt, fps[0:H, :], AF.Exp,
                                     scale=1.0 - NEG_SLOPE)
                nc.sync.dma_start(out=dT_dram, in_=dTt)
                bcast(0)
                bcast(1)
                bcast(2)
                bcast(3)
                # ec_all cols: [0:8]=E2, [8:16]=E2s, [16:24]=-E2s
                nc.scalar.activation(ec_all[:, :, 0:H], fcols, AF.Exp,
                                     scale=1.0)
                nc.scalar.activation(ec_all[:, :, H:2 * H], fcols, AF.Exp,
                                     scale=NEG_SLOPE)
                nc.vector.tensor_scalar(ec_all[:, :, 2 * H:3 * H],
                                        ec_all[:, :, H:2 * H], -1.0, None,
                                        op0=OP.mult)
            ecols = [ec_all[:, it, :] for it in range(NIT)]

            # deferred: wrs (Wh inputs) + adj tiles, then later broadcasts
            for kt in range(NKT):
                nc.scalar.dma_start(out=wrs_sb[:, kt, :],
                                    in_=wrsb[kt * P:(kt + 1) * P, :])
            for eng, jt in ((nc.sync, 0), (nc.scalar, 1), (nc.sync, 2),
                            (nc.scalar, 3)):
                eng.dma_start(out=adj_all[:, jt, :],
                              in_=adjT[jt * P:(jt + 1) * P, :])
            bcast(4)
            bcast(5)
            for eng, jt in ((nc.sync, 4), (nc.scalar, 5)):
                eng.dma_start(out=adj_all[:, jt, :],
                              in_=adjT[jt * P:(jt + 1) * P, :])
            bcast(6)
            bcast(7)
            for eng, jt in ((nc.sync, 6), (nc.scalar, 7)):
                eng.dma_start(out=adj_all[:, jt, :],
                              in_=adjT[jt * P:(jt + 1) * P, :])
            # broadcast-view reads of adj_all do not register DMA deps;
            # anchor each tile with a tiny in-order DVE read before the
            # first masked use (pair 0 emits the masks first)
            adjdep = ecolp.tile([P, NJT], bf16)

            # ---- Phase 2: Wh (bf16) with aug ones column -------------------
            for it in range(NIT):
                ps2 = ps_misc.tile([P, H, D_OUT], f32, tag="m",
                                   name=f"ps2_{it}")
                for kt in range(NKT):
                    lhsT = htb_sb[:, kt, it * P:(it + 1) * P]
                    nc.tensor.matmul(ps2, lhsT, wrs_sb[:, kt, :],
                                     start=(kt == 0), stop=(kt == NKT - 1))
                nc.scalar.copy(whaug[it][:, :, 0:D_OUT], ps2)

            out_big = whp.tile([P, NIT, HF], bf16)

            # ---- Phase 3: per head-pair attention --------------------------
            def _fin_transpose_norm(h0, ots_pair, pot=None):
                for k in range(2):
                    h = h0 + k
                    ots = ots_pair[k]
                    if pot is not None:
                        nc.scalar.copy(ots[0:AUG, :], pot[k])
                    trs = fin.tile([P, NIT, TRW], bf16, tag="trs",
                                   bufs=2, name="trs")
                    nc.sync.dma_start_transpose(out=trs, in_=ots[0:TRW, :])
                    rc = fin.tile([P, NIT, 1], f32, tag="rc", bufs=2,
                                  name="rc")
                    nc.vector.reciprocal(rc[:, 0:4, :], trs[:, 0:4, 64:65])
                    nc.vector.reciprocal(rc[:, 4:8, :], trs[:, 4:8, 64:65])
                    for g in range(2):
                        src = trs[:, g * 4:(g + 1) * 4, 0:D_OUT]
                        rcb = rc[:, g * 4:(g + 1) * 4, :] \
                            .broadcast_to([P, 4, D_OUT])
                        dst = out_big[:, g * 4:(g + 1) * 4,
                                      h * D_OUT:(h + 1) * D_OUT]
                        nc.gpsimd.tensor_tensor(out=dst, in0=src,
                                                in1=rcb, op=OP.mult)


            def _out_dma(h0):
                orr = out.rearrange("(it p) c -> p it c", p=P)
                for k, eng in ((0, nc.sync), (1, nc.scalar)):
                    csl = slice((h0 + k) * D_OUT, (h0 + k + 1) * D_OUT)
                    eng.dma_start(out=orr[:, :, csl],
                                  in_=out_big[:, :, csl])

            def ts_unit(dst, jt, h):
                nc.vector.tensor_scalar(
                    dst, dbc_all[:, h, :],
                    ecols[jt][:, h:h + 1],
                    ecols[jt][:, H + h:H + h + 1],
                    op0=OP.mult, op1=OP.max)

            def relu_unit(r_dst, tq_dst, jt, h):
                nc.scalar.activation(
                    r_dst, dbc_all[:, h, :], AF.Relu,
                    bias=ecols[jt][:, 2 * H + h:2 * H + h + 1],
                    scale=ecols[jt][:, h:h + 1])
                nc.scalar.activation(
                    tq_dst, r_dst, AF.Relu,
                    bias=ecols[jt][:, H + h:H + h + 1],
                    scale=1.0)

            pending = None
            for p in range(NPAIR):
                h0 = 2 * p
                bset = B_UNITS[p]

                ot = [ps_ot.tile([AUG, N], f32, tag="ot", name=f"ot{k}")
                      for k in range(2)]

                # two mega tiles: jt0-3 and jt4-7.  Pair 0 runs the TS
                # and masks in column halves so compute starts as soon
                # as the first half of each d-broadcast lands (the dbc
                # DMA is the gating transfer); matmul nh-splits line up.
                tqA = work.tile([P, 2, 4, N], bf16, tag="tqA", name="tqA")
                tqB = work.tile([P, 2, 4, N], bf16, tag="tqB", name="tqB")
                HALVES = (slice(0, 512), slice(512, 1024)) \
                    if p == 0 else (slice(0, 1024),)

                def ts_unit_h(dst, jt, h, sl):
                    nc.vector.tensor_scalar(
                        dst[:, sl], dbc_all[:, h, sl],
                        ecols[jt][:, h:h + 1],
                        ecols[jt][:, H + h:H + h + 1],
                        op0=OP.mult, op1=OP.max)

                # ACT: B-lane units (two passes each)
                for (jt, k) in bset:
                    h = h0 + k
                    r = work.tile([P, N], bf16, tag=f"r{jt}{k}",
                                  name=f"r{jt}{k}")
                    relu_unit(r, tqB[:, k, jt - 4, :], jt, h)

                # DVE: TS for all D units, then the mega mask TTs
                for sl in HALVES:
                    for k in range(2):
                        for jt in range(4):
                            ts_unit_h(tqA[:, k, jt, :], jt, h0 + k, sl)
                umA = ump.tile([P, 2, 4, N], bf16, tag="umA", name="umA")
                if p == 0:
                    for jt in range(4):
                        nc.vector.tensor_copy(adjdep[:, jt:jt + 1],
                                              adj_all[:, jt, 0:1])
                for sl in HALVES:
                    adjA = adj_all[:, 0:4, sl].unsqueeze(1) \
                        .broadcast_to([P, 2, 4, sl.stop - sl.start])
                    nc.vector.tensor_tensor(out=umA[:, :, :, sl],
                                            in0=tqA[:, :, :, sl],
                                            in1=adjA, op=OP.mult)
                for sl in HALVES:
                    for jt in range(4, NJT):
                        for k in range(2):
                            if (jt, k) not in bset:
                                ts_unit_h(tqB[:, k, jt - 4, :], jt,
                                          h0 + k, sl)
                umB = ump.tile([P, 2, 4, N], bf16, tag="umB", name="umB")
                if p == 0:
                    for jt in range(4, NJT):
                        nc.vector.tensor_copy(adjdep[:, jt:jt + 1],
                                              adj_all[:, jt, 0:1])
                for sl in HALVES:
                    adjB = adj_all[:, 4:8, sl].unsqueeze(1) \
                        .broadcast_to([P, 2, 4, sl.stop - sl.start])
                    nc.vector.tensor_tensor(out=umB[:, :, :, sl],
                                            in0=tqB[:, :, :, sl],
                                            in1=adjB, op=OP.mult)

                # Pool: evac prev pair's PSUM, then normalize prev pair;
                # out DMA deferred one more pair to keep SP free for
                # the latency-critical transposes
                if pending is not None:
                    ph0, pot, pots = pending
                    with tc.high_priority(offset=2000):
                        _fin_transpose_norm(ph0, pots, pot)
                if p >= 2:
                    _out_dma(2 * (p - 2))

                # PE: two 8-matmul bursts (full-width rhs), gated on one
                # mega mask each; last pair k-major so ot[0] stops early
                def mm_one(gum, jt, k):
                    lhsT = whaug[jt][:, h0 + k, :]
                    for nh in range(2):
                        nc.tensor.matmul(
                            ot[k][:, nh * 512:(nh + 1) * 512], lhsT,
                            gum[:, k, jt % 4, nh * 512:(nh + 1) * 512],
                            start=(jt == 0), stop=(jt == NJT - 1))

                for gum, lo in ((umA, 0), (umB, 4)):
                    for jt in range(lo, lo + 4):
                        for k in range(2):
                            mm_one(gum, jt, k)

                ots_pair = [ots_tiles[(2 * p + k) % 4] for k in range(2)]
                pending = (h0, ot, ots_pair)

            # drain: pair-2 out, last pair finalize, last out
            _out_dma(4)
            ph0, pot, pots = pending
            _fin_transpose_norm(ph0, pots, pot)
            _out_dma(6)

    nc.compile()
    return nc


def _host_prep(h, adj, W, a):
    a1, a2 = a[:, :D_OUT], a[:, D_OUT:]
    w1 = np.einsum("hdf,hf->hd", W, a1).astype(np.float32)
    w2 = np.einsum("hdf,hf->hd", W, a2).astype(np.float32)
    w12 = np.concatenate(
        [w2.T, np.zeros((D_IN, 24), np.float32), w1.T], axis=1).astype(BF16)
    wrs = np.ascontiguousarray(
        W.transpose(1, 0, 2).reshape(D_IN, HF)).astype(BF16)
    in_maps = []
    for b in range(B):
        in_maps.append({
            "hTb": np.ascontiguousarray(h[b].T).astype(BF16),
            "adjT": np.ascontiguousarray(adj[b].T).astype(BF16),
            "wrsb": wrs,
            "w12": w12,
        })
    return in_maps


def kernel(h, adj, W, a):
    from concourse.bass_utils import run_bass_kernel_spmd

    in_maps = _host_prep(np.asarray(h), np.asarray(adj),
                         np.asarray(W), np.asarray(a))
    nc = _build_program()
    res = run_bass_kernel_spmd(nc, in_maps, core_ids=list(range(B)))
    out = np.stack([np.asarray(res.results[b]["out"]) for b in range(B)])
    return out.astype(np.float32)


# revision 41
# speedup vs baseline: 1.0331x; 1.0331x over previous
"""Multi-head graph attention layer (GAT) for Trainium2, 8-core data-parallel.

Problem: B=8, N=1024, D_IN=256, D_OUT=64, H=8, LeakyReLU slope 0.2.
Sharding: one batch element per NeuronCore.

Algebra: with x = f1_i + f2_j and exp monotone, the unnormalized softmax
weight (after factoring out exp(0.2 f1_i), which cancels) is
  U[j,i] = adj[j,i] * max(d_i * E2_j, E2s_j)
with d = exp(0.8 f1), E2 = exp(f2), E2s = exp(0.2 f2).
out^T = [Wh|1]^T @ U gives numerators + the denominator row Z; the
finalize transposes via the DMA XBAR and normalizes.

Measured op costs (ns, on throttled shared hw): DVE TS [P,1024] ~540,
DVE mask TT ~550/unit (mega-quad [P,2,4,N]), ACT pass ~1100-1230, Pool
TT [P,2,N] ~4300 (useless for big tiles).  Hence: all masks on DVE
mega-quads; B lanes (2 ACT passes, jt4/5 + jt6k0) only where DVE
saturates; Pool does only the small normalize TTs.  Pair 0 is all-D so
ACT pre-computes pair 1's B tiles during it (pipeline fill).

Everything is bf16 (one h load; scores f32-accumulated on PE).  f2 is
computed directly as COLUMNS via tiny per-node-tile matmuls (no XBAR
transposes in the prologue); d rows bounce via DRAM and are partition-
broadcast in split halves across both HWDGE queues at high priority,
ahead of the bulk adj loads (the DMA fabric is the prologue
bottleneck).  Finalize (PSUM evac -> XBAR transpose -> reciprocal ->
normalize -> per-pair output DMA) is pipelined one pair behind and
runs at high priority so it interleaves with the loop instead of
serializing at the end.  Broadcast-view reads (mask TTs) do not
register DMA dependencies in the tile framework; tiny anchor copies on
the DVE queue order each adj tile before its first masked use.
"""

import numpy as np
import ml_dtypes

BF16 = ml_dtypes.bfloat16

B, N, D_IN, D_OUT, H = 8, 1024, 256, 64, 8
NEG_SLOPE = 0.2
P = 128
NJT = N // P                  # 8 j tiles
NIT = N // P                  # 8 i tiles
NKT = D_IN // P               # 2 contraction tiles
HF = H * D_OUT                # 512
AUG = D_OUT + 1               # 65
TRW = 80                      # transpose row count (65 padded to %16)
NPAIR = H // 2
# w12 cols: [0:8]=w2; w1_h0 at col 32 (score row 0), w1_h1 at col 64
# (score row 32) so both d rows sit at legal PE base partitions for the
# ones-matmul broadcast; w1_h2-7 at cols 33-38 (rows 1-6, DMA bounce)
W12C = 97

# B-lane units per pair (pair 0 all-D): jt4,jt5 both k, jt6 k0
B_UNITS = {1: ((4, 0), (4, 1), (5, 0), (5, 1), (6, 0)),
           2: ((4, 0), (4, 1), (5, 0), (5, 1), (6, 0)),
           3: ((4, 0), (4, 1), (5, 0), (5, 1), (6, 0)),
           0: ()}


def _build_program():
    import concourse.bass as bass
    import concourse.bacc as bacc
    import concourse.tile as tile
    from concourse import mybir

    f32 = mybir.dt.float32
    bf16 = mybir.dt.bfloat16
    AF = mybir.ActivationFunctionType
    OP = mybir.AluOpType

    nc = bacc.Bacc("TRN2", target_bir_lowering=False, debug=False,
                   enable_asserts=False, num_devices=8)

    hTb = nc.dram_tensor("hTb", [D_IN, N], bf16, kind="ExternalInput").ap()
    adjT = nc.dram_tensor("adjT", [N, N], bf16, kind="ExternalInput").ap()
    wrsb = nc.dram_tensor("wrsb", [D_IN, HF], bf16,
                          kind="ExternalInput").ap()
    w12 = nc.dram_tensor("w12", [D_IN, W12C], bf16,
                         kind="ExternalInput").ap()
    out = nc.dram_tensor("out", [N, HF], bf16, kind="ExternalOutput").ap()

    with tile.TileContext(nc) as tc:
        with (
            tc.tile_pool(name="const", bufs=1) as const,
            tc.tile_pool(name="inputs", bufs=1) as inputs,
            tc.tile_pool(name="whp", bufs=1) as whp,
            tc.tile_pool(name="ecol", bufs=1) as ecolp,
            tc.tile_pool(name="ps_f", bufs=1, space="PSUM") as ps_f,
            tc.tile_pool(name="ps_misc", bufs=1, space="PSUM") as ps_misc,
            tc.tile_pool(name="ps_ot", bufs=2, space="PSUM") as ps_ot,
            tc.tile_pool(name="work", bufs=1) as work,
            tc.tile_pool(name="ump", bufs=2) as ump,
            tc.tile_pool(name="fin", bufs=2) as fin,
            tc.tile_pool(name="dram", bufs=1, space="DRAM") as dramp,
        ):
            # ---- Phase 0: DMA issue ----------------------------------------
            # SP: score/Wh inputs first, then adj3/4 while dTt pends
            htb_sb = inputs.tile([P, NKT, N], bf16)
            w12_sb = inputs.tile([P, NKT, W12C], bf16)
            wrs_sb = inputs.tile([P, NKT, HF], bf16)
            adj_all = inputs.tile([P, NJT, N], bf16)
            for kt in range(NKT):
                nc.sync.dma_start(out=htb_sb[:, kt, :],
                                  in_=hTb[kt * P:(kt + 1) * P, :])
            for kt in range(NKT):
                nc.scalar.dma_start(out=w12_sb[:, kt, :],
                                    in_=w12[kt * P:(kt + 1) * P, :])

            # Pool: warmup operand + whaug ones + persistent evac targets
            z512 = const.tile([P, 512], bf16)
            nc.gpsimd.memset(z512, 0.0)
            whaug = []
            for it in range(NIT):
                wa = whp.tile([P, H, AUG], bf16, tag=f"whaug{it}",
                              name=f"whaug{it}")
                nc.gpsimd.memset(wa[:, :, D_OUT], 1.0)
                whaug.append(wa)
            ots_tiles = []
            for i in range(4):
                t = whp.tile([TRW, N], bf16, tag=f"ots{i}", name=f"ots{i}")
                nc.gpsimd.memset(t[D_OUT:TRW, :], 0.0)
                ots_tiles.append(t)

            # ones rows for the PE d-broadcast (bases 0 and 32)
            ones33 = const.tile([33, P], bf16)
            nc.gpsimd.memset(ones33[0:1, :], 1.0)
            nc.gpsimd.memset(ones33[32:33, :], 1.0)

            # ---- PE warmup chain (p-state ramp), reusing the score PSUM ----
            FW = 65
            fps = ps_f.tile([FW, N], f32)
            NWARM = 7
            for i in range(NWARM):
                nc.tensor.matmul(fps[:, 0:512], z512[:, 0:FW], z512,
                                 start=(i == 0), stop=(i == NWARM - 1))

            # ---- Phase 1: f1 score rows (w1 block of w12) ------------------
            # row 0 = f1 h0, row 32 = f1 h1, rows 1-6 = f1 h2-7
            for half in range(2):
                sl = slice(half * 512, (half + 1) * 512)
                for kt in range(NKT):
                    nc.tensor.matmul(fps[:, sl], w12_sb[:, kt, 32:32 + FW],
                                     htb_sb[:, kt, sl],
                                     start=(kt == 0), stop=(kt == NKT - 1))

            # ---- f2 as columns: tiny matmuls, node tiles stationary --------
            fcols = ps_f.tile([P, NIT, H], f32, tag="fc")
            for it in range(NIT):
                for kt in range(NKT):
                    nc.tensor.matmul(fcols[:, it, :],
                                     htb_sb[:, kt, it * P:(it + 1) * P],
                                     w12_sb[:, kt, 0:H],
                                     start=(kt == 0), stop=(kt == NKT - 1))

            # d = exp(0.8 f1) row form -> DRAM bounce -> partition
            # broadcast; E2/E2s straight from the f2 COLUMNS (no XBAR
            # transposes).  This chain gates the first attention op: pin
            # it at high priority.
            dTt = ecolp.tile([FW, N], bf16)
            dT_dram = dramp.tile([H - 2, N], bf16)
            dbc_all = ecolp.tile([P, H, N], bf16)
            ec_all = ecolp.tile([P, NIT, 3 * H], f32)

            def bcast(h, eng=None):
                if h < 2:
                    # PE ones-matmul broadcast into PSUM + ACT evac:
                    # skips the DMA fabric for the loop-gating heads
                    row = 32 * h
                    for half in range(2):
                        sl = slice(half * 512, (half + 1) * 512)
                        pbc = ps_misc.tile([P, 512], f32, tag="m",
                                           name=f"pbc{h}{half}")
                        nc.tensor.matmul(pbc, ones33[row:row + 1, :],
                                         dTt[row:row + 1, sl],
                                         start=True, stop=True)
                        nc.scalar.copy(dbc_all[:, h, sl], pbc)
                    return
                src_ap = dT_dram[h - 2:h - 1, :]
                nc.sync.dma_start(
                    out=dbc_all[:, h, 0:512],
                    in_=src_ap[:, 0:512].partition_broadcast(P))
                nc.scalar.dma_start(
                    out=dbc_all[:, h, 512:1024],
                    in_=src_ap[:, 512:1024].partition_broadcast(P))

            with tc.high_priority():
                nc.scalar.activation(dTt, fps, AF.Exp,
                                     scale=1.0 - NEG_SLOPE)
                bcast(0)
                bcast(1)
                nc.sync.dma_start(out=dT_dram, in_=dTt[1:1 + H - 2, :])
                bcast(2)
                bcast(3)
                # ec_all cols: [0:8]=E2, [8:16]=E2s, [16:24]=-E2s
                nc.scalar.activation(ec_all[:, :, 0:H], fcols, AF.Exp,
                                     scale=1.0)
                nc.scalar.activation(ec_all[:, :, H:2 * H], fcols, AF.Exp,
                                     scale=NEG_SLOPE)
                nc.vector.tensor_scalar(ec_all[:, :, 2 * H:3 * H],
                                        ec_all[:, :, H:2 * H], -1.0, None,
                                        op0=OP.mult)
            ecols = [ec_all[:, it, :] for it in range(NIT)]

            # deferred: wrs (Wh inputs) + adj tiles, then later broadcasts
            for kt in range(NKT):
                nc.scalar.dma_start(out=wrs_sb[:, kt, :],
                                    in_=wrsb[kt * P:(kt + 1) * P, :])
            for eng, jt in ((nc.sync, 0), (nc.scalar, 1), (nc.sync, 2),
                            (nc.scalar, 3)):
                eng.dma_start(out=adj_all[:, jt, :],
                              in_=adjT[jt * P:(jt + 1) * P, :])
            bcast(4)
            bcast(5)
            for eng, jt in ((nc.sync, 4), (nc.scalar, 5)):
                eng.dma_start(out=adj_all[:, jt, :],
                              in_=adjT[jt * P:(jt + 1) * P, :])
            bcast(6)
            bcast(7)
            for eng, jt in ((nc.sync, 6), (nc.scalar, 7)):
                eng.dma_start(out=adj_all[:, jt, :],
                              in_=adjT[jt * P:(jt + 1) * P, :])
            # broadcast-view reads of adj_all do not register DMA deps;
            # anchor each tile with a tiny in-order DVE read before the
            # first masked use (pair 0 emits the masks first)
            adjdep = ecolp.tile([P, NJT], bf16)

            # ---- Phase 2: Wh (bf16) with aug ones column -------------------
            for it in range(NIT):
                ps2 = ps_misc.tile([P, H, D_OUT], f32, tag="m",
                                   name=f"ps2_{it}")
                for kt in range(NKT):
                    lhsT = htb_sb[:, kt, it * P:(it + 1) * P]
                    nc.tensor.matmul(ps2, lhsT, wrs_sb[:, kt, :],
                                     start=(kt == 0), stop=(kt == NKT - 1))
                nc.scalar.copy(whaug[it][:, :, 0:D_OUT], ps2)

            out_big = whp.tile([P, NIT, HF], bf16)

            # ---- Phase 3: per head-pair attention --------------------------
            def _fin_transpose_norm(h0, ots_pair, pot=None):
                for k in range(2):
                    h = h0 + k
                    ots = ots_pair[k]
                    if pot is not None:
                        nc.scalar.copy(ots[0:AUG, :], pot[k])
                    trs = fin.tile([P, NIT, TRW], bf16, tag="trs",
                                   bufs=2, name="trs")
                    nc.sync.dma_start_transpose(out=trs, in_=ots[0:TRW, :])
                    rc = fin.tile([P, NIT, 1], f32, tag="rc", bufs=2,
                                  name="rc")
                    nc.vector.reciprocal(rc[:, 0:4, :], trs[:, 0:4, 64:65])
                    nc.vector.reciprocal(rc[:, 4:8, :], trs[:, 4:8, 64:65])
                    for g in range(2):
                        src = trs[:, g * 4:(g + 1) * 4, 0:D_OUT]
                        rcb = rc[:, g * 4:(g + 1) * 4, :] \
                            .broadcast_to([P, 4, D_OUT])
                        dst = out_big[:, g * 4:(g + 1) * 4,
                                      h * D_OUT:(h + 1) * D_OUT]
                        nc.gpsimd.tensor_tensor(out=dst, in0=src,
                                                in1=rcb, op=OP.mult)


            def _out_dma(h0):
                orr = out.rearrange("(it p) c -> p it c", p=P)
                for k, eng in ((0, nc.sync), (1, nc.scalar)):
                    csl = slice((h0 + k) * D_OUT, (h0 + k + 1) * D_OUT)
                    eng.dma_start(out=orr[:, :, csl],
                                  in_=out_big[:, :, csl])

            def ts_unit(dst, jt, h):
                nc.vector.tensor_scalar(
                    dst, dbc_all[:, h, :],
                    ecols[jt][:, h:h + 1],
                    ecols[jt][:, H + h:H + h + 1],
                    op0=OP.mult, op1=OP.max)

            def relu_unit(r_dst, tq_dst, jt, h):
                nc.scalar.activation(
                    r_dst, dbc_all[:, h, :], AF.Relu,
                    bias=ecols[jt][:, 2 * H + h:2 * H + h + 1],
                    scale=ecols[jt][:, h:h + 1])
                nc.scalar.activation(
                    tq_dst, r_dst, AF.Relu,
                    bias=ecols[jt][:, H + h:H + h + 1],
                    scale=1.0)

            pending = None
            for p in range(NPAIR):
                h0 = 2 * p
                bset = B_UNITS[p]

                ot = [ps_ot.tile([AUG, N], f32, tag="ot", name=f"ot{k}")
                      for k in range(2)]

                # two mega tiles: jt0-3 and jt4-7.  Pair 0 runs the TS
                # and masks in column halves so compute starts as soon
                # as the first half of each d-broadcast lands (the dbc
                # DMA is the gating transfer); matmul nh-splits line up.
                tqA = work.tile([P, 2, 4, N], bf16, tag="tqA", name="tqA")
                tqB = work.tile([P, 2, 4, N], bf16, tag="tqB", name="tqB")
                HALVES = (slice(0, 512), slice(512, 1024)) \
                    if p == 0 else (slice(0, 1024),)

                def ts_unit_h(dst, jt, h, sl):
                    nc.vector.tensor_scalar(
                        dst[:, sl], dbc_all[:, h, sl],
                        ecols[jt][:, h:h + 1],
                        ecols[jt][:, H + h:H + h + 1],
                        op0=OP.mult, op1=OP.max)

                # ACT: B-lane units (two passes each)
                for (jt, k) in bset:
                    h = h0 + k
                    r = work.tile([P, N], bf16, tag=f"r{jt}{k}",
                                  name=f"r{jt}{k}")
                    relu_unit(r, tqB[:, k, jt - 4, :], jt, h)

                # DVE: TS for all D units, then the mega mask TTs
                for sl in HALVES:
                    for k in range(2):
                        for jt in range(4):
                            ts_unit_h(tqA[:, k, jt, :], jt, h0 + k, sl)
                umA = ump.tile([P, 2, 4, N], bf16, tag="umA", name="umA")
                if p == 0:
                    for jt in range(4):
                        nc.vector.tensor_copy(adjdep[:, jt:jt + 1],
                                              adj_all[:, jt, 0:1])
                for sl in HALVES:
                    adjA = adj_all[:, 0:4, sl].unsqueeze(1) \
                        .broadcast_to([P, 2, 4, sl.stop - sl.start])
                    nc.vector.tensor_tensor(out=umA[:, :, :, sl],
                                            in0=tqA[:, :, :, sl],
                                            in1=adjA, op=OP.mult)
                for sl in HALVES:
                    for jt in range(4, NJT):
                        for k in range(2):
                            if (jt, k) not in bset:
                                ts_unit_h(tqB[:, k, jt - 4, :], jt,
                                          h0 + k, sl)
                umB = ump.tile([P, 2, 4, N], bf16, tag="umB", name="umB")
                if p == 0:
                    for jt in range(4, NJT):
                        nc.vector.tensor_copy(adjdep[:, jt:jt + 1],
                                              adj_all[:, jt, 0:1])
                for sl in HALVES:
                    adjB = adj_all[:, 4:8, sl].unsqueeze(1) \
                        .broadcast_to([P, 2, 4, sl.stop - sl.start])
                    nc.vector.tensor_tensor(out=umB[:, :, :, sl],
                                            in0=tqB[:, :, :, sl],
                                            in1=adjB, op=OP.mult)

                # Pool: evac prev pair's PSUM, then normalize prev pair;
                # out DMA deferred one more pair to keep SP free for
                # the latency-critical transposes
                if pending is not None:
                    ph0, pot, pots = pending
                    with tc.high_priority(offset=2000):
                        _fin_transpose_norm(ph0, pots, pot)
                if p >= 2:
                    _out_dma(2 * (p - 2))

                # PE: two 8-matmul bursts (full-width rhs), gated on one
                # mega mask each; last pair k-major so ot[0] stops early
                def mm_one(gum, jt, k):
                    lhsT = whaug[jt][:, h0 + k, :]
                    for nh in range(2):
                        nc.tensor.matmul(
                            ot[k][:, nh * 512:(nh + 1) * 512], lhsT,
                            gum[:, k, jt % 4, nh * 512:(nh + 1) * 512],
                            start=(jt == 0), stop=(jt == NJT - 1))

                for gum, lo in ((umA, 0), (umB, 4)):
                    for jt in range(lo, lo + 4):
                        for k in range(2):
                            mm_one(gum, jt, k)

                ots_pair = [ots_tiles[(2 * p + k) % 4] for k in range(2)]
                pending = (h0, ot, ots_pair)

            # drain: pair-2 out, last pair finalize, last out
            _out_dma(4)
            ph0, pot, pots = pending
            _fin_transpose_norm(ph0, pots, pot)
            _out_dma(6)

    nc.compile()
    return nc


def _host_prep(h, adj, W, a):
    a1, a2 = a[:, :D_OUT], a[:, D_OUT:]
    w1 = np.einsum("hdf,hf->hd", W, a1).astype(np.float32)
    w2 = np.einsum("hdf,hf->hd", W, a2).astype(np.float32)
    w12 = np.zeros((D_IN, W12C), np.float32)
    w12[:, 0:H] = w2.T
    w12[:, 32] = w1[0]
    w12[:, 64] = w1[1]
    for hh in range(2, H):
        w12[:, 33 + hh - 2] = w1[hh]
    w12 = w12.astype(BF16)
    wrs = np.ascontiguousarray(
        W.transpose(1, 0, 2).reshape(D_IN, HF)).astype(BF16)
    in_maps = []
    for b in range(B):
        in_maps.append({
            "hTb": np.ascontiguousarray(h[b].T).astype(BF16),
            "adjT": np.ascontiguousarray(adj[b].T).astype(BF16),
            "wrsb": wrs,
            "w12": w12,
        })
    return in_maps


def kernel(h, adj, W, a):
    from concourse.bass_utils import run_bass_kernel_spmd

    in_maps = _host_prep(np.asarray(h), np.asarray(adj),
                         np.asarray(W), np.asarray(a))
    nc = _build_program()
    res = run_bass_kernel_spmd(nc, in_maps, core_ids=list(range(B)))
    out = np.stack([np.asarray(res.results[b]["out"]) for b in range(B)])
    return out.astype(np.float32)
